# revision 6
# baseline (speedup 1.0000x reference)
"""AssistedExcitation Trainium2 kernel.

out[b,c,h,w] = x[b,c,h,w] + bbox_mask[b,h,w] * mean_c(x[b,:,h,w])

Data-parallel over 8 NeuronCores: 2 images per core, no collectives.
HBM I/O in bf16 (rel-err budget 2e-2 >> bf16 rounding ~3e-3): halves
DMA traffic vs f32 -> ~94us/core roofline at 358 GB/s.

Per core, per [256, 4096] chunk (channel halves A/B on partitions):
  - channel sums via matmul with a 1/256 bf16 column into [1,1024]
    PSUM pair tiles,
  - DVE mul with the flat [1,HW] bf16 bbox mask -> masked means (ad),
  - K=1 broadcast matmuls spread ad across 128 partitions (PSUM),
    grouped after all sums so the PE runs long same-stationary streaks
    (TRN2 PE p-state: full 2.4 GHz only after ~3us continuous busy),
  - ACT copies PSUM->SBUF bf16, DVE does two fused 4096-wide bf16 adds
    (all-SBUF bf16 step-1 => 2x DVE packing mode),
  - chunk pipeline is software-staggered: adds/stores of chunk c-1 are
    emitted after the front half of chunk c, keeping every in-order
    engine queue free of cross-engine round-trip stalls.
Preamble (box rasterization) reads one packed [128,146] const DMA on
the scalar ring so it never queues behind 1 MiB x loads on sync.
"""

import sys

sys.path.insert(0, "/opt/trn_rl_repo")

import ml_dtypes
import numpy as np

import concourse.bacc as bacc
import concourse.bass as bass
import concourse.mybir as mybir
import concourse.tile as tile
from concourse import bass_utils

# Problem constants (hardcoded per harness contract)
B, C, H, W = 16, 256, 128, 128
N_BOX = 320
N_CORES = 8
B_SHARD = B // N_CORES  # 2 images per core
HW = H * W  # 16384
P = 128  # partitions
CHUNK = 4096  # free-dim elements per x tile (32 rows of the image)
N_CHUNK = HW // CHUNK  # 4
SUB = 512  # matmul moving free-dim (one PSUM bank of f32)
PAIR = 2 * SUB  # 1024: one [1, PAIR] PSUM sum tile = 2 banks
N_PAIR = CHUNK // PAIR  # 4
NBOX_PAD = 384  # 320 boxes padded to 3 tiles of 128
N_BOX_TILES = NBOX_PAD // P  # 3
ALPHA = 1.0
# packed const layout: [iota(128) | boxes(3*4) | sel(3*2)]
CONST_COLS = P + 4 * N_BOX_TILES + 2 * N_BOX_TILES  # 146

F32 = mybir.dt.float32
BF16 = mybir.dt.bfloat16


def build_nc():
    """Build the per-core Bass graph (SPMD: same graph on all 8 cores)."""
    nc = bacc.Bacc(None, target_bir_lowering=False)

    x = nc.declare_dram_parameter("x", [B_SHARD, C, HW], BF16, isOutput=False)
    consts = nc.declare_dram_parameter("consts", [P, CONST_COLS], F32, isOutput=False)
    out = nc.declare_dram_parameter("out", [B_SHARD, C, HW], BF16, isOutput=True)

    with tile.TileContext(nc) as tc:
        with (
            tc.tile_pool(name="const", bufs=1) as constp,
            tc.tile_pool(name="boxp", bufs=1) as boxp,
            tc.tile_pool(name="maskp", bufs=1) as maskp,
            tc.tile_pool(name="xp", bufs=3) as xp,
            tc.tile_pool(name="outp", bufs=2) as outp,
            tc.tile_pool(name="pbsp", bufs=2) as pbsp,
            tc.tile_pool(name="adp", bufs=4) as adp,
            tc.tile_pool(name="smallp", bufs=4) as smallp,
            tc.tile_pool(name="ps_s", bufs=2, space=bass.MemorySpace.PSUM) as ps_s,
            tc.tile_pool(name="ps_b", bufs=3, space=bass.MemorySpace.PSUM) as ps_b,
            tc.tile_pool(name="ps_m", bufs=1, space=bass.MemorySpace.PSUM) as ps_m,
        ):
            # --- constants: one small DMA on the scalar ring ---
            cst = constp.tile([P, CONST_COLS], F32)
            nc.scalar.dma_start(cst[:], consts[:])
            iota_f = cst[:, 0:P]
            wsum = constp.tile([P, 1], BF16)  # 1/C column -> channel mean
            nc.vector.memset(wsum[:], ALPHA / C)
            ones1 = constp.tile([1, P], BF16)  # K=1 broadcast row
            nc.vector.memset(ones1[:], 1.0)

            # --- box rasterization setup (tiny) ---
            # Per box n (on partitions): vx1m1 = (xc-bw/2)*W - 1, vx2 = (xc+bw/2)*W
            # cols[n,w] = (w > vx1m1) & (w <= vx2)   (== ref's clamped-int test)
            # valid = (#cols>=2) & (#rows>=2)        (== ref's x2>x1 & y2>y1)
            rows_sel = [[None] * N_BOX_TILES for _ in range(B_SHARD)]
            cols_val = [None] * N_BOX_TILES
            rows_raw, sel_tiles = [], []
            for t in range(N_BOX_TILES):
                bx = cst[:, P + 4 * t : P + 4 * (t + 1)]
                st = cst[:, P + 4 * N_BOX_TILES + 2 * t : P + 4 * N_BOX_TILES + 2 * (t + 1)]

                xc, yc, bw, bh = (bx[:, i : i + 1] for i in range(4))
                hbw = smallp.tile([P, 1], F32, tag="hbw")
                nc.vector.tensor_scalar_mul(hbw[:], bw, 0.5)
                hbh = smallp.tile([P, 1], F32, tag="hbh")
                nc.vector.tensor_scalar_mul(hbh[:], bh, 0.5)

                def edge(center, half, w_scale, bias, tag):
                    lo = smallp.tile([P, 1], F32, tag=tag + "a")
                    nc.vector.tensor_tensor(
                        lo[:], center, half[:],
                        op=mybir.AluOpType.subtract if bias else mybir.AluOpType.add,
                    )
                    o = smallp.tile([P, 1], F32, tag=tag + "b")
                    if bias:
                        nc.vector.tensor_scalar(
                            o[:], lo[:], float(w_scale), -1.0,
                            op0=mybir.AluOpType.mult, op1=mybir.AluOpType.add,
                        )
                    else:
                        nc.vector.tensor_scalar_mul(o[:], lo[:], float(w_scale))
                    return o

                vx1m1 = edge(xc, hbw, W, True, "vx1")
                vx2 = edge(xc, hbw, W, False, "vx2")
                vy1m1 = edge(yc, hbh, H, True, "vy1")
                vy2 = edge(yc, hbh, H, False, "vy2")

                def member(lo_m1, hi, tag):
                    g1 = smallp.tile([P, P], F32, tag=tag + "g1")
                    nc.vector.tensor_scalar(
                        g1[:], iota_f, lo_m1[:], None, op0=mybir.AluOpType.is_gt
                    )
                    g2 = smallp.tile([P, P], F32, tag=tag + "g2")
                    nc.vector.tensor_scalar(
                        g2[:], iota_f, hi[:], None, op0=mybir.AluOpType.is_le
                    )
                    m = boxp.tile([P, P], F32, tag=tag + "m")
                    nc.vector.tensor_mul(m[:], g1[:], g2[:])
                    return m

                cols = member(vx1m1, vx2, f"c{t}")
                rows = member(vy1m1, vy2, f"r{t}")

                def count_ok(m, tag):
                    cnt = smallp.tile([P, 1], F32, tag=tag + "cnt")
                    nc.vector.tensor_reduce(
                        cnt[:], m[:], axis=mybir.AxisListType.X, op=mybir.AluOpType.add
                    )
                    ok = smallp.tile([P, 1], F32, tag=tag + "ok")
                    nc.vector.tensor_scalar(
                        ok[:], cnt[:], 1.5, None, op0=mybir.AluOpType.is_ge
                    )
                    return ok

                cok = count_ok(cols, f"c{t}")
                rok = count_ok(rows, f"r{t}")
                vfac = smallp.tile([P, 1], F32, tag="vfac")
                nc.vector.tensor_mul(vfac[:], cok[:], rok[:])

                cv = boxp.tile([P, P], F32, tag=f"cv{t}")
                nc.vector.tensor_scalar(
                    cv[:], cols[:], vfac[:], None, op0=mybir.AluOpType.mult
                )
                cols_val[t] = cv
                rows_raw.append(rows)
                sel_tiles.append(st)

            # --- per-image mask -> flat [1, HW] bf16 on partition 0 via a
            # small HWDGE flatten on the scalar ring. Image 0 first so the
            # main stream unblocks early.
            mflat = []
            for j in range(B_SHARD):
                for t in range(N_BOX_TILES):
                    rs = boxp.tile([P, P], F32, tag=f"rs{t}_{j}")
                    nc.vector.tensor_scalar(
                        rs[:], rows_raw[t][:], sel_tiles[t][:, j : j + 1], None,
                        op0=mybir.AluOpType.mult,
                    )
                    rows_sel[j][t] = rs
                pm = ps_m.tile([P, W], F32)
                for t in range(N_BOX_TILES):
                    nc.tensor.matmul(
                        pm[:], rows_sel[j][t][:], cols_val[t][:],
                        start=(t == 0), stop=(t == N_BOX_TILES - 1),
                    )
                msb = maskp.tile([P, W], BF16, tag=f"msb{j}")
                nc.vector.tensor_scalar_min(msb[:], pm[:], 1.0)
                mf = maskp.tile([1, HW], BF16, tag=f"mf{j}")
                nc.scalar.dma_start(mf[:], msb[:])
                mflat.append(mf)

            # --- main stream: 8 chunks of [256, 4096], software-pipelined ---
            def emit_front(b, ci):
                csl = slice(ci * CHUNK, (ci + 1) * CHUNK)
                A = xp.tile([P, CHUNK], BF16, tag="A")
                nc.sync.dma_start(A[:], x[b, 0:P, csl])
                Bt = xp.tile([P, CHUNK], BF16, tag="B")
                nc.sync.dma_start(Bt[:], x[b, P:C, csl])
                # channel sums: one long same-stationary PE streak
                pss, ads = [], []
                for sp in range(N_PAIR):
                    ps = ps_s.tile([1, PAIR], F32)
                    for h in range(2):
                        ssl = slice((sp * 2 + h) * SUB, (sp * 2 + h + 1) * SUB)
                        hsl = slice(h * SUB, (h + 1) * SUB)
                        nc.tensor.matmul(
                            ps[:, hsl], wsum[:], A[:, ssl], start=True, stop=False
                        )
                        nc.tensor.matmul(
                            ps[:, hsl], wsum[:], Bt[:, ssl], start=False, stop=True
                        )
                    pss.append(ps)
                    # masked means (frees the ps slot for pair sp+2)
                    ad = adp.tile([1, PAIR], BF16, tag="ad")
                    moff = ci * CHUNK + sp * PAIR
                    nc.vector.tensor_mul(
                        ad[:], ps[:], mflat[b][0:1, moff : moff + PAIR]
                    )
                    ads.append(ad)
                # broadcasts: second same-stationary PE streak; ACT converts
                pbs = pbsp.tile([P, CHUNK], BF16, tag="pbs")
                for sp in range(N_PAIR):
                    for h in range(2):
                        pb = ps_b.tile([P, SUB], F32)
                        nc.tensor.matmul(
                            pb[:], ones1[:], ads[sp][:, h * SUB : (h + 1) * SUB],
                            start=True, stop=True,
                        )
                        psl = slice((sp * 2 + h) * SUB, (sp * 2 + h + 1) * SUB)
                        nc.scalar.copy(pbs[:, psl], pb[:])
                return (b, ci, A, Bt, pbs)

            def emit_back(st):
                b, ci, A, Bt, pbs = st
                csl = slice(ci * CHUNK, (ci + 1) * CHUNK)
                oA = outp.tile([P, CHUNK], BF16, tag="oA")
                oB = outp.tile([P, CHUNK], BF16, tag="oB")
                nc.vector.tensor_add(oA[:], A[:], pbs[:])
                nc.vector.tensor_add(oB[:], Bt[:], pbs[:])
                nc.sync.dma_start(out[b, 0:P, csl], oA[:])
                nc.scalar.dma_start(out[b, P:C, csl], oB[:])

            prev = None
            for b in range(B_SHARD):
                for ci in range(N_CHUNK):
                    cur = emit_front(b, ci)
                    if prev is not None:
                        emit_back(prev)
                    prev = cur
            emit_back(prev)

    return nc


def _host_prep(x, bboxes, batch_idx):
    """Shard inputs; cast x to bf16; build the packed const array."""
    x = (
        np.ascontiguousarray(np.asarray(x, dtype=np.float32))
        .reshape(B, C, HW)
        .astype(ml_dtypes.bfloat16)
    )
    bboxes = np.asarray(bboxes, dtype=np.float32)
    batch_idx = np.asarray(batch_idx).astype(np.int64)

    boxes_pad = np.zeros((NBOX_PAD, 4), dtype=np.float32)
    boxes_pad[:N_BOX] = bboxes
    # [128, 12]: partition p, tile t -> box t*128+p
    boxes_cols = boxes_pad.reshape(N_BOX_TILES, P, 4).transpose(1, 0, 2).reshape(P, -1)
    iota = np.broadcast_to(np.arange(P, dtype=np.float32), (P, P))

    in_maps = []
    for i in range(N_CORES):
        sel_i = np.zeros((NBOX_PAD, 2), dtype=np.float32)
        for j in range(B_SHARD):
            sel_i[:N_BOX, j] = (batch_idx == (i * B_SHARD + j)).astype(np.float32)
        sel_cols = sel_i.reshape(N_BOX_TILES, P, 2).transpose(1, 0, 2).reshape(P, -1)
        consts = np.concatenate([iota, boxes_cols, sel_cols], axis=1).astype(np.float32)
        in_maps.append(
            {
                "x": np.ascontiguousarray(x[i * B_SHARD : (i + 1) * B_SHARD]),
                "consts": np.ascontiguousarray(consts),
            }
        )
    return in_maps


def kernel(x, bboxes, batch_idx):
    in_maps = _host_prep(x, bboxes, batch_idx)
    nc = build_nc()
    nc.finalize()
    res = bass_utils.run_bass_kernel_spmd(nc, in_maps, core_ids=list(range(N_CORES)))
    shards = [
        np.asarray(res.results[i]["out"]).astype(np.float32) for i in range(N_CORES)
    ]
    return np.concatenate(shards, axis=0).reshape(B, C, H, W)


if __name__ == "__main__":
    nc = build_nc()
    nc.finalize()
    print("built ok:", len(nc.inst_map), "instructions")


# revision 7
# speedup vs baseline: 1.0159x; 1.0159x over previous
"""AssistedExcitation Trainium2 kernel.

out[b,c,h,w] = x[b,c,h,w] + bbox_mask[b,h,w] * mean_c(x[b,:,h,w])

Data-parallel over 8 NeuronCores: 2 images per core, no collectives.
HBM I/O in bf16 (rel-err budget 2e-2 >> bf16 rounding ~3e-3): halves
DMA traffic vs f32 -> ~94us/core roofline at 358 GB/s.

Per core, per [256, 4096] chunk (channel halves A/B on partitions):
  - channel sums via matmul with a 1/256 bf16 column into [1,1024]
    PSUM pair tiles,
  - DVE mul with the flat [1,HW] bf16 bbox mask -> masked means (ad),
  - K=1 broadcast matmuls spread ad across 128 partitions (PSUM),
    grouped after all sums so the PE runs long same-stationary streaks
    (TRN2 PE p-state: full 2.4 GHz only after ~3us continuous busy),
  - ACT copies PSUM->SBUF bf16, DVE does two fused 4096-wide bf16 adds
    (all-SBUF bf16 step-1 => 2x DVE packing mode),
  - chunk pipeline is software-staggered: adds/stores of chunk c-1 are
    emitted after the front half of chunk c, keeping every in-order
    engine queue free of cross-engine round-trip stalls.
Preamble (box rasterization) reads one packed [128,146] const DMA on
the scalar ring so it never queues behind 1 MiB x loads on sync.
"""

import sys

sys.path.insert(0, "/opt/trn_rl_repo")

import ml_dtypes
import numpy as np

import concourse.bacc as bacc
import concourse.bass as bass
import concourse.mybir as mybir
import concourse.tile as tile
from concourse import bass_utils

# Problem constants (hardcoded per harness contract)
B, C, H, W = 16, 256, 128, 128
N_BOX = 320
N_CORES = 8
B_SHARD = B // N_CORES  # 2 images per core
HW = H * W  # 16384
P = 128  # partitions
CHUNK = 4096  # free-dim elements per x tile (32 rows of the image)
N_CHUNK = HW // CHUNK  # 4
SUB = 512  # matmul moving free-dim (one PSUM bank of f32)
PAIR = 2 * SUB  # 1024: one [1, PAIR] PSUM sum tile = 2 banks
N_PAIR = CHUNK // PAIR  # 4
NBOX_PAD = 384  # 320 boxes padded to 3 tiles of 128
N_BOX_TILES = NBOX_PAD // P  # 3
ALPHA = 1.0
# packed const layout: [iota(128) | boxes(3*4) | sel(3*2)]
CONST_COLS = P + 4 * N_BOX_TILES + 2 * N_BOX_TILES  # 146

F32 = mybir.dt.float32
BF16 = mybir.dt.bfloat16


def build_nc():
    """Build the per-core Bass graph (SPMD: same graph on all 8 cores)."""
    nc = bacc.Bacc(None, target_bir_lowering=False)

    x = nc.declare_dram_parameter("x", [B_SHARD, C, HW], BF16, isOutput=False)
    consts = nc.declare_dram_parameter("consts", [P, CONST_COLS], F32, isOutput=False)
    out = nc.declare_dram_parameter("out", [B_SHARD, C, HW], BF16, isOutput=True)

    with tile.TileContext(nc) as tc:
        with (
            tc.tile_pool(name="const", bufs=1) as constp,
            tc.tile_pool(name="boxp", bufs=1) as boxp,
            tc.tile_pool(name="maskp", bufs=1) as maskp,
            tc.tile_pool(name="xp", bufs=3) as xp,
            tc.tile_pool(name="outp", bufs=2) as outp,
            tc.tile_pool(name="pbsp", bufs=2) as pbsp,
            tc.tile_pool(name="adp", bufs=4) as adp,
            tc.tile_pool(name="smallp", bufs=4) as smallp,
            tc.tile_pool(name="ps_s", bufs=2, space=bass.MemorySpace.PSUM) as ps_s,
            tc.tile_pool(name="ps_b", bufs=3, space=bass.MemorySpace.PSUM) as ps_b,
            tc.tile_pool(name="ps_m", bufs=1, space=bass.MemorySpace.PSUM) as ps_m,
        ):
            # --- constants: one small DMA on the scalar ring ---
            cst = constp.tile([P, CONST_COLS], F32)
            nc.scalar.dma_start(cst[:], consts[:])
            iota_f = cst[:, 0:P]
            wsum = constp.tile([P, 1], BF16)  # 1/C column -> channel mean
            nc.vector.memset(wsum[:], ALPHA / C)
            ones1 = constp.tile([1, P], BF16)  # K=1 broadcast row
            nc.vector.memset(ones1[:], 1.0)

            # --- box rasterization setup (tiny) ---
            # Per box n (on partitions): vx1m1 = (xc-bw/2)*W - 1, vx2 = (xc+bw/2)*W
            # cols[n,w] = (w > vx1m1) & (w <= vx2)   (== ref's clamped-int test)
            # valid = (#cols>=2) & (#rows>=2)        (== ref's x2>x1 & y2>y1)
            rows_sel = [[None] * N_BOX_TILES for _ in range(B_SHARD)]
            cols_val = [None] * N_BOX_TILES
            rows_raw, sel_tiles = [], []
            for t in range(N_BOX_TILES):
                bx = cst[:, P + 4 * t : P + 4 * (t + 1)]
                st = cst[:, P + 4 * N_BOX_TILES + 2 * t : P + 4 * N_BOX_TILES + 2 * (t + 1)]

                xc, yc, bw, bh = (bx[:, i : i + 1] for i in range(4))
                hbw = smallp.tile([P, 1], F32, tag="hbw")
                nc.vector.tensor_scalar_mul(hbw[:], bw, 0.5)
                hbh = smallp.tile([P, 1], F32, tag="hbh")
                nc.vector.tensor_scalar_mul(hbh[:], bh, 0.5)

                def edge(center, half, w_scale, bias, tag):
                    lo = smallp.tile([P, 1], F32, tag=tag + "a")
                    nc.vector.tensor_tensor(
                        lo[:], center, half[:],
                        op=mybir.AluOpType.subtract if bias else mybir.AluOpType.add,
                    )
                    o = smallp.tile([P, 1], F32, tag=tag + "b")
                    if bias:
                        nc.vector.tensor_scalar(
                            o[:], lo[:], float(w_scale), -1.0,
                            op0=mybir.AluOpType.mult, op1=mybir.AluOpType.add,
                        )
                    else:
                        nc.vector.tensor_scalar_mul(o[:], lo[:], float(w_scale))
                    return o

                vx1m1 = edge(xc, hbw, W, True, "vx1")
                vx2 = edge(xc, hbw, W, False, "vx2")
                vy1m1 = edge(yc, hbh, H, True, "vy1")
                vy2 = edge(yc, hbh, H, False, "vy2")

                def member(lo_m1, hi, tag):
                    g1 = smallp.tile([P, P], F32, tag=tag + "g1")
                    nc.vector.tensor_scalar(
                        g1[:], iota_f, lo_m1[:], None, op0=mybir.AluOpType.is_gt
                    )
                    g2 = smallp.tile([P, P], F32, tag=tag + "g2")
                    nc.vector.tensor_scalar(
                        g2[:], iota_f, hi[:], None, op0=mybir.AluOpType.is_le
                    )
                    m = boxp.tile([P, P], F32, tag=tag + "m")
                    nc.vector.tensor_mul(m[:], g1[:], g2[:])
                    return m

                cols = member(vx1m1, vx2, f"c{t}")
                rows = member(vy1m1, vy2, f"r{t}")

                def count_ok(m, tag):
                    cnt = smallp.tile([P, 1], F32, tag=tag + "cnt")
                    nc.vector.tensor_reduce(
                        cnt[:], m[:], axis=mybir.AxisListType.X, op=mybir.AluOpType.add
                    )
                    ok = smallp.tile([P, 1], F32, tag=tag + "ok")
                    nc.vector.tensor_scalar(
                        ok[:], cnt[:], 1.5, None, op0=mybir.AluOpType.is_ge
                    )
                    return ok

                cok = count_ok(cols, f"c{t}")
                rok = count_ok(rows, f"r{t}")
                vfac = smallp.tile([P, 1], F32, tag="vfac")
                nc.vector.tensor_mul(vfac[:], cok[:], rok[:])

                cv = boxp.tile([P, P], F32, tag=f"cv{t}")
                nc.vector.tensor_scalar(
                    cv[:], cols[:], vfac[:], None, op0=mybir.AluOpType.mult
                )
                cols_val[t] = cv
                rows_raw.append(rows)
                sel_tiles.append(st)

            # --- per-image mask -> flat [1, HW] bf16 on partition 0 via a
            # small HWDGE flatten on the scalar ring. Image 0 first so the
            # main stream unblocks early.
            mflat = []
            for j in range(B_SHARD):
                for t in range(N_BOX_TILES):
                    rs = boxp.tile([P, P], F32, tag=f"rs{t}_{j}")
                    nc.vector.tensor_scalar(
                        rs[:], rows_raw[t][:], sel_tiles[t][:, j : j + 1], None,
                        op0=mybir.AluOpType.mult,
                    )
                    rows_sel[j][t] = rs
                pm = ps_m.tile([P, W], F32)
                for t in range(N_BOX_TILES):
                    nc.tensor.matmul(
                        pm[:], rows_sel[j][t][:], cols_val[t][:],
                        start=(t == 0), stop=(t == N_BOX_TILES - 1),
                    )
                msb = maskp.tile([P, W], BF16, tag=f"msb{j}")
                nc.vector.tensor_scalar_min(msb[:], pm[:], 1.0)
                mf = maskp.tile([1, HW], BF16, tag=f"mf{j}")
                nc.scalar.dma_start(mf[:], msb[:])
                mflat.append(mf)

            # --- main stream: 8 chunks of [256, 4096], software-pipelined.
            # One 2 MiB load per chunk on the sync HWDGE ring (loads only:
            # no store waits can head-of-line-block the prefetch); one 2 MiB
            # store per chunk via SWDGE on the Pool queue (store waits only
            # block other stores). X/O tiles are [128, 2*CHUNK]: columns
            # 0:CHUNK = channels 0-127, CHUNK:2*CHUNK = channels 128-255.
            def emit_front(b, ci):
                csl = slice(ci * CHUNK, (ci + 1) * CHUNK)
                X = xp.tile([P, 2 * CHUNK], BF16, tag="X")
                nc.sync.dma_start(
                    X[:], x[b, :, csl].rearrange("(h p) w -> p h w", h=2)
                )
                # channel sums: one long same-stationary PE streak
                pss, ads = [], []
                for sp in range(N_PAIR):
                    ps = ps_s.tile([1, PAIR], F32)
                    for h in range(2):
                        ssl = slice((sp * 2 + h) * SUB, (sp * 2 + h + 1) * SUB)
                        bsl = slice(
                            CHUNK + (sp * 2 + h) * SUB, CHUNK + (sp * 2 + h + 1) * SUB
                        )
                        hsl = slice(h * SUB, (h + 1) * SUB)
                        nc.tensor.matmul(
                            ps[:, hsl], wsum[:], X[:, ssl], start=True, stop=False
                        )
                        nc.tensor.matmul(
                            ps[:, hsl], wsum[:], X[:, bsl], start=False, stop=True
                        )
                    pss.append(ps)
                    # masked means (frees the ps slot for pair sp+2)
                    ad = adp.tile([1, PAIR], BF16, tag="ad")
                    moff = ci * CHUNK + sp * PAIR
                    nc.vector.tensor_mul(
                        ad[:], ps[:], mflat[b][0:1, moff : moff + PAIR]
                    )
                    ads.append(ad)
                # broadcasts: second same-stationary PE streak; ACT converts
                pbs = pbsp.tile([P, CHUNK], BF16, tag="pbs")
                for sp in range(N_PAIR):
                    for h in range(2):
                        pb = ps_b.tile([P, SUB], F32)
                        nc.tensor.matmul(
                            pb[:], ones1[:], ads[sp][:, h * SUB : (h + 1) * SUB],
                            start=True, stop=True,
                        )
                        psl = slice((sp * 2 + h) * SUB, (sp * 2 + h + 1) * SUB)
                        nc.scalar.copy(pbs[:, psl], pb[:])
                return (b, ci, X, pbs)

            def emit_back(st):
                b, ci, X, pbs = st
                csl = slice(ci * CHUNK, (ci + 1) * CHUNK)
                O = outp.tile([P, 2 * CHUNK], BF16, tag="O")
                nc.vector.tensor_add(O[:, 0:CHUNK], X[:, 0:CHUNK], pbs[:])
                nc.vector.tensor_add(O[:, CHUNK : 2 * CHUNK], X[:, CHUNK : 2 * CHUNK], pbs[:])
                nc.gpsimd.dma_start(
                    out[b, :, csl].rearrange("(h p) w -> p h w", h=2), O[:]
                )

            prev = None
            for b in range(B_SHARD):
                for ci in range(N_CHUNK):
                    cur = emit_front(b, ci)
                    if prev is not None:
                        emit_back(prev)
                    prev = cur
            emit_back(prev)

    return nc


def _host_prep(x, bboxes, batch_idx):
    """Shard inputs; cast x to bf16; build the packed const array."""
    x = (
        np.ascontiguousarray(np.asarray(x, dtype=np.float32))
        .reshape(B, C, HW)
        .astype(ml_dtypes.bfloat16)
    )
    bboxes = np.asarray(bboxes, dtype=np.float32)
    batch_idx = np.asarray(batch_idx).astype(np.int64)

    boxes_pad = np.zeros((NBOX_PAD, 4), dtype=np.float32)
    boxes_pad[:N_BOX] = bboxes
    # [128, 12]: partition p, tile t -> box t*128+p
    boxes_cols = boxes_pad.reshape(N_BOX_TILES, P, 4).transpose(1, 0, 2).reshape(P, -1)
    iota = np.broadcast_to(np.arange(P, dtype=np.float32), (P, P))

    in_maps = []
    for i in range(N_CORES):
        sel_i = np.zeros((NBOX_PAD, 2), dtype=np.float32)
        for j in range(B_SHARD):
            sel_i[:N_BOX, j] = (batch_idx == (i * B_SHARD + j)).astype(np.float32)
        sel_cols = sel_i.reshape(N_BOX_TILES, P, 2).transpose(1, 0, 2).reshape(P, -1)
        consts = np.concatenate([iota, boxes_cols, sel_cols], axis=1).astype(np.float32)
        in_maps.append(
            {
                "x": np.ascontiguousarray(x[i * B_SHARD : (i + 1) * B_SHARD]),
                "consts": np.ascontiguousarray(consts),
            }
        )
    return in_maps


def kernel(x, bboxes, batch_idx):
    in_maps = _host_prep(x, bboxes, batch_idx)
    nc = build_nc()
    nc.finalize()
    res = bass_utils.run_bass_kernel_spmd(nc, in_maps, core_ids=list(range(N_CORES)))
    shards = [
        np.asarray(res.results[i]["out"]).astype(np.float32) for i in range(N_CORES)
    ]
    return np.concatenate(shards, axis=0).reshape(B, C, H, W)


if __name__ == "__main__":
    nc = build_nc()
    nc.finalize()
    print("built ok:", len(nc.inst_map), "instructions")


# revision 11
# speedup vs baseline: 1.1369x; 1.1191x over previous
"""AssistedExcitation Trainium2 kernel.

out[b,c,h,w] = x[b,c,h,w] + bbox_mask[b,h,w] * mean_c(x[b,:,h,w])

Data-parallel over 8 NeuronCores: 2 images per core, no collectives.
HBM I/O in bf16 (rel-err budget 2e-2 >> bf16 rounding ~3e-3): halves
DMA traffic vs f32 -> ~94us/core roofline at 358 GB/s.

Per core, per [256, 4096] chunk (channel halves A/B on partitions):
  - channel sums via matmul with a 1/256 bf16 column into [1,1024]
    PSUM pair tiles,
  - DVE mul with the flat [1,HW] bf16 bbox mask -> masked means (ad),
  - K=1 broadcast matmuls spread ad across 128 partitions (PSUM),
    grouped after all sums so the PE runs long same-stationary streaks
    (TRN2 PE p-state: full 2.4 GHz only after ~3us continuous busy),
  - ACT copies PSUM->SBUF bf16, DVE does two fused 4096-wide bf16 adds
    (all-SBUF bf16 step-1 => 2x DVE packing mode),
  - chunk pipeline is software-staggered: adds/stores of chunk c-1 are
    emitted after the front half of chunk c, keeping every in-order
    engine queue free of cross-engine round-trip stalls.
Preamble (box rasterization) reads one packed [128,146] const DMA on
the scalar ring so it never queues behind 1 MiB x loads on sync.
"""

import sys

sys.path.insert(0, "/opt/trn_rl_repo")

import ml_dtypes
import numpy as np

import concourse.bacc as bacc
import concourse.bass as bass
import concourse.mybir as mybir
import concourse.tile as tile
from concourse import bass_utils

# Problem constants (hardcoded per harness contract)
B, C, H, W = 16, 256, 128, 128
N_BOX = 320
N_CORES = 8
B_SHARD = B // N_CORES  # 2 images per core
HW = H * W  # 16384
P = 128  # partitions
CHUNK = 4096  # free-dim elements per x tile (32 rows of the image)
N_CHUNK = HW // CHUNK  # 4
SUB = 512  # matmul moving free-dim (one PSUM bank of f32)
PAIR = 2 * SUB  # 1024: one [1, PAIR] PSUM sum tile = 2 banks
N_PAIR = CHUNK // PAIR  # 4
NBOX_PAD = 384  # 320 boxes padded to 3 tiles of 128
N_BOX_TILES = NBOX_PAD // P  # 3
ALPHA = 1.0
# packed const layout: [iota(128) | boxes(3*4) | sel(3*2)]
CONST_COLS = P + 4 * N_BOX_TILES + 2 * N_BOX_TILES  # 146

F32 = mybir.dt.float32
BF16 = mybir.dt.bfloat16


def build_nc():
    """Build the per-core Bass graph (SPMD: same graph on all 8 cores)."""
    nc = bacc.Bacc(None, target_bir_lowering=False)

    x = nc.declare_dram_parameter("x", [B_SHARD, C, HW], BF16, isOutput=False)
    consts = nc.declare_dram_parameter("consts", [P, CONST_COLS], F32, isOutput=False)
    out = nc.declare_dram_parameter("out", [B_SHARD, C, HW], BF16, isOutput=True)

    with tile.TileContext(nc) as tc:
        with (
            tc.tile_pool(name="const", bufs=1) as constp,
            tc.tile_pool(name="boxp", bufs=1) as boxp,
            tc.tile_pool(name="maskp", bufs=1) as maskp,
            tc.tile_pool(name="xp", bufs=4) as xp,
            tc.tile_pool(name="outp", bufs=2) as outp,
            tc.tile_pool(name="pbsp", bufs=2) as pbsp,
            tc.tile_pool(name="adp", bufs=2) as adp,
            tc.tile_pool(name="smallp", bufs=2) as smallp,
            tc.tile_pool(name="ps_s", bufs=2, space=bass.MemorySpace.PSUM) as ps_s,
            tc.tile_pool(name="ps_b", bufs=3, space=bass.MemorySpace.PSUM) as ps_b,
            tc.tile_pool(name="ps_m", bufs=1, space=bass.MemorySpace.PSUM) as ps_m,
        ):
            # --- constants: one small DMA on the scalar ring ---
            cst = constp.tile([P, CONST_COLS], F32)
            nc.scalar.dma_start(cst[:], consts[:])
            iota_f = cst[:, 0:P]
            wsum = constp.tile([P, 1], BF16)  # 1/C column -> channel mean
            nc.vector.memset(wsum[:], ALPHA / C)
            ones1 = constp.tile([1, P], BF16)  # K=1 broadcast row
            nc.vector.memset(ones1[:], 1.0)

            # --- box rasterization setup (tiny) ---
            # Per box n (on partitions): vx1m1 = (xc-bw/2)*W - 1, vx2 = (xc+bw/2)*W
            # cols[n,w] = (w > vx1m1) & (w <= vx2)   (== ref's clamped-int test)
            # valid = (#cols>=2) & (#rows>=2)        (== ref's x2>x1 & y2>y1)
            rows_sel = [[None] * N_BOX_TILES for _ in range(B_SHARD)]
            cols_val = [None] * N_BOX_TILES
            rows_raw, sel_tiles = [], []
            for t in range(N_BOX_TILES):
                # box tiles are independent: run tile 1 on the idle Pool
                # engine so the mask critical path is ~DVE-time/2
                eng = nc.gpsimd if t == 1 else nc.vector
                bx = cst[:, P + 4 * t : P + 4 * (t + 1)]
                st = cst[:, P + 4 * N_BOX_TILES + 2 * t : P + 4 * N_BOX_TILES + 2 * (t + 1)]

                xc, yc, bw, bh = (bx[:, i : i + 1] for i in range(4))
                hbw = smallp.tile([P, 1], F32, tag=f"hbw{t}")
                eng.tensor_scalar_mul(hbw[:], bw, 0.5)
                hbh = smallp.tile([P, 1], F32, tag=f"hbh{t}")
                eng.tensor_scalar_mul(hbh[:], bh, 0.5)

                def edge(center, half, w_scale, bias, tag):
                    lo = smallp.tile([P, 1], F32, tag=tag + "a")
                    eng.tensor_tensor(
                        lo[:], center, half[:],
                        op=mybir.AluOpType.subtract if bias else mybir.AluOpType.add,
                    )
                    o = smallp.tile([P, 1], F32, tag=tag + "b")
                    if bias:
                        eng.tensor_scalar(
                            o[:], lo[:], float(w_scale), -1.0,
                            op0=mybir.AluOpType.mult, op1=mybir.AluOpType.add,
                        )
                    else:
                        eng.tensor_scalar_mul(o[:], lo[:], float(w_scale))
                    return o

                vx1m1 = edge(xc, hbw, W, True, f"vx1{t}")
                vx2 = edge(xc, hbw, W, False, f"vx2{t}")
                vy1m1 = edge(yc, hbh, H, True, f"vy1{t}")
                vy2 = edge(yc, hbh, H, False, f"vy2{t}")

                def member(lo_m1, hi, tag):
                    g1 = smallp.tile([P, P], F32, tag=tag + "g1")
                    eng.tensor_scalar(
                        g1[:], iota_f, lo_m1[:], None, op0=mybir.AluOpType.is_gt
                    )
                    g2 = smallp.tile([P, P], F32, tag=tag + "g2")
                    eng.tensor_scalar(
                        g2[:], iota_f, hi[:], None, op0=mybir.AluOpType.is_le
                    )
                    m = boxp.tile([P, P], F32, tag=tag + "m")
                    eng.tensor_mul(m[:], g1[:], g2[:])
                    return m

                cols = member(vx1m1, vx2, f"c{t}")
                rows = member(vy1m1, vy2, f"r{t}")

                def count_ok(m, tag):
                    # free-axis reduce is DVE-only (GpSimd lacks it)
                    cnt = smallp.tile([P, 1], F32, tag=tag + "cnt")
                    nc.vector.tensor_reduce(
                        cnt[:], m[:], axis=mybir.AxisListType.X, op=mybir.AluOpType.add
                    )
                    ok = smallp.tile([P, 1], F32, tag=tag + "ok")
                    nc.vector.tensor_scalar(
                        ok[:], cnt[:], 1.5, None, op0=mybir.AluOpType.is_ge
                    )
                    return ok

                cok = count_ok(cols, f"c{t}")
                rok = count_ok(rows, f"r{t}")
                vfac = smallp.tile([P, 1], F32, tag=f"vfac{t}")
                eng.tensor_mul(vfac[:], cok[:], rok[:])

                cv = boxp.tile([P, P], F32, tag=f"cv{t}")
                eng.tensor_scalar(
                    cv[:], cols[:], vfac[:], None, op0=mybir.AluOpType.mult
                )
                cols_val[t] = cv
                rows_raw.append(rows)
                sel_tiles.append(st)

            # --- per-image mask -> flat [1, HW] bf16 on partition 0 via a
            # small HWDGE flatten on the scalar ring. Image 0 first so the
            # main stream unblocks early.
            mflat = []
            for j in range(B_SHARD):
                for t in range(N_BOX_TILES):
                    rs = boxp.tile([P, P], F32, tag=f"rs{t}_{j}")
                    reng = nc.gpsimd if t == 1 else nc.vector
                    reng.tensor_scalar(
                        rs[:], rows_raw[t][:], sel_tiles[t][:, j : j + 1], None,
                        op0=mybir.AluOpType.mult,
                    )
                    rows_sel[j][t] = rs
                pm = ps_m.tile([P, W], F32)
                for t in range(N_BOX_TILES):
                    nc.tensor.matmul(
                        pm[:], rows_sel[j][t][:], cols_val[t][:],
                        start=(t == 0), stop=(t == N_BOX_TILES - 1),
                    )
                msb = maskp.tile([P, W], BF16, tag=f"msb{j}")
                nc.vector.tensor_scalar_min(msb[:], pm[:], 1.0)
                mf = maskp.tile([1, HW], BF16, tag=f"mf{j}")
                nc.scalar.dma_start(mf[:], msb[:])
                mflat.append(mf)

            # --- main stream: 8 chunks of [256, 4096], software-pipelined.
            # One 2 MiB load per chunk on the sync HWDGE ring (loads only:
            # no store waits can head-of-line-block the prefetch); one 2 MiB
            # store per chunk via SWDGE on the Pool queue (store waits only
            # block other stores). X/O tiles are [128, 2*CHUNK]: columns
            # 0:CHUNK = channels 0-127, CHUNK:2*CHUNK = channels 128-255.
            def emit_front(b, ci):
                csl = slice(ci * CHUNK, (ci + 1) * CHUNK)
                X = xp.tile([P, 2 * CHUNK], BF16, tag="X")
                nc.sync.dma_start(
                    X[:], x[b, :, csl].rearrange("(h p) w -> p h w", h=2)
                )
                # channel sums: one long same-stationary PE streak
                pss, ads = [], []
                for sp in range(N_PAIR):
                    ps = ps_s.tile([1, PAIR], F32)
                    for h in range(2):
                        ssl = slice((sp * 2 + h) * SUB, (sp * 2 + h + 1) * SUB)
                        bsl = slice(
                            CHUNK + (sp * 2 + h) * SUB, CHUNK + (sp * 2 + h + 1) * SUB
                        )
                        hsl = slice(h * SUB, (h + 1) * SUB)
                        nc.tensor.matmul(
                            ps[:, hsl], wsum[:], X[:, ssl], start=True, stop=False
                        )
                        nc.tensor.matmul(
                            ps[:, hsl], wsum[:], X[:, bsl], start=False, stop=True
                        )
                    pss.append(ps)
                    # masked means (frees the ps slot for pair sp+2)
                    ad = adp.tile([1, PAIR], BF16, tag="ad")
                    moff = ci * CHUNK + sp * PAIR
                    nc.vector.tensor_mul(
                        ad[:], ps[:], mflat[b][0:1, moff : moff + PAIR]
                    )
                    ads.append(ad)
                # broadcasts: second same-stationary PE streak; ACT converts
                pbs = pbsp.tile([P, CHUNK], BF16, tag="pbs")
                for sp in range(N_PAIR):
                    for h in range(2):
                        pb = ps_b.tile([P, SUB], F32)
                        nc.tensor.matmul(
                            pb[:], ones1[:], ads[sp][:, h * SUB : (h + 1) * SUB],
                            start=True, stop=True,
                        )
                        psl = slice((sp * 2 + h) * SUB, (sp * 2 + h + 1) * SUB)
                        nc.scalar.copy(pbs[:, psl], pb[:])
                return (b, ci, X, pbs)

            def emit_back(st):
                b, ci, X, pbs = st
                csl = slice(ci * CHUNK, (ci + 1) * CHUNK)
                O = outp.tile([P, 2 * CHUNK], BF16, tag="O")
                nc.vector.tensor_add(O[:, 0:CHUNK], X[:, 0:CHUNK], pbs[:])
                nc.vector.tensor_add(O[:, CHUNK : 2 * CHUNK], X[:, CHUNK : 2 * CHUNK], pbs[:])
                nc.gpsimd.dma_start(
                    out[b, :, csl].rearrange("(h p) w -> p h w", h=2), O[:]
                )

            prev = None
            for b in range(B_SHARD):
                for ci in range(N_CHUNK):
                    cur = emit_front(b, ci)
                    if prev is not None:
                        emit_back(prev)
                    prev = cur
            emit_back(prev)

    return nc


def _host_prep(x, bboxes, batch_idx):
    """Shard inputs; cast x to bf16; build the packed const array."""
    x = (
        np.ascontiguousarray(np.asarray(x, dtype=np.float32))
        .reshape(B, C, HW)
        .astype(ml_dtypes.bfloat16)
    )
    bboxes = np.asarray(bboxes, dtype=np.float32)
    batch_idx = np.asarray(batch_idx).astype(np.int64)

    boxes_pad = np.zeros((NBOX_PAD, 4), dtype=np.float32)
    boxes_pad[:N_BOX] = bboxes
    # [128, 12]: partition p, tile t -> box t*128+p
    boxes_cols = boxes_pad.reshape(N_BOX_TILES, P, 4).transpose(1, 0, 2).reshape(P, -1)
    iota = np.broadcast_to(np.arange(P, dtype=np.float32), (P, P))

    in_maps = []
    for i in range(N_CORES):
        sel_i = np.zeros((NBOX_PAD, 2), dtype=np.float32)
        for j in range(B_SHARD):
            sel_i[:N_BOX, j] = (batch_idx == (i * B_SHARD + j)).astype(np.float32)
        sel_cols = sel_i.reshape(N_BOX_TILES, P, 2).transpose(1, 0, 2).reshape(P, -1)
        consts = np.concatenate([iota, boxes_cols, sel_cols], axis=1).astype(np.float32)
        in_maps.append(
            {
                "x": np.ascontiguousarray(x[i * B_SHARD : (i + 1) * B_SHARD]),
                "consts": np.ascontiguousarray(consts),
            }
        )
    return in_maps


def kernel(x, bboxes, batch_idx):
    in_maps = _host_prep(x, bboxes, batch_idx)
    nc = build_nc()
    nc.finalize()
    res = bass_utils.run_bass_kernel_spmd(nc, in_maps, core_ids=list(range(N_CORES)))
    shards = [
        np.asarray(res.results[i]["out"]).astype(np.float32) for i in range(N_CORES)
    ]
    return np.concatenate(shards, axis=0).reshape(B, C, H, W)


if __name__ == "__main__":
    nc = build_nc()
    nc.finalize()
    print("built ok:", len(nc.inst_map), "instructions")


# revision 13
# speedup vs baseline: 1.1606x; 1.0209x over previous
"""AssistedExcitation Trainium2 kernel.

out[b,c,h,w] = x[b,c,h,w] + bbox_mask[b,h,w] * mean_c(x[b,:,h,w])

Data-parallel over 8 NeuronCores: 2 images per core, no collectives.
HBM I/O in bf16 (rel-err budget 2e-2 >> bf16 rounding ~3e-3): halves
DMA traffic vs f32 -> ~94us/core roofline at 358 GB/s.

Per core, per [256, 4096] chunk (channel halves A/B on partitions):
  - channel sums via matmul with a 1/256 bf16 column into [1,1024]
    PSUM pair tiles,
  - DVE mul with the flat [1,HW] bf16 bbox mask -> masked means (ad),
  - K=1 broadcast matmuls spread ad across 128 partitions (PSUM),
    grouped after all sums so the PE runs long same-stationary streaks
    (TRN2 PE p-state: full 2.4 GHz only after ~3us continuous busy),
  - ACT copies PSUM->SBUF bf16, DVE does two fused 4096-wide bf16 adds
    (all-SBUF bf16 step-1 => 2x DVE packing mode),
  - chunk pipeline is software-staggered: adds/stores of chunk c-1 are
    emitted after the front half of chunk c, keeping every in-order
    engine queue free of cross-engine round-trip stalls.
Preamble (box rasterization) reads one packed [128,146] const DMA on
the scalar ring so it never queues behind 1 MiB x loads on sync.
"""

import sys

sys.path.insert(0, "/opt/trn_rl_repo")

import ml_dtypes
import numpy as np

import concourse.bacc as bacc
import concourse.bass as bass
import concourse.mybir as mybir
import concourse.tile as tile
from concourse import bass_utils

# Problem constants (hardcoded per harness contract)
B, C, H, W = 16, 256, 128, 128
N_BOX = 320
N_CORES = 8
B_SHARD = B // N_CORES  # 2 images per core
HW = H * W  # 16384
P = 128  # partitions
CHUNK = 4096  # free-dim elements per x tile (32 rows of the image)
N_CHUNK = HW // CHUNK  # 4
SUB = 512  # matmul moving free-dim (one PSUM bank of f32)
PAIR = 2 * SUB  # 1024: one [1, PAIR] PSUM sum tile = 2 banks
N_PAIR = CHUNK // PAIR  # 4
NBOX_PAD = 384  # 320 boxes padded to 3 tiles of 128
N_BOX_TILES = NBOX_PAD // P  # 3
ALPHA = 1.0
# packed const layout: [iota(128) | boxes(3*4) | sel(3*2)]
CONST_COLS = P + 4 * N_BOX_TILES + 2 * N_BOX_TILES  # 146

F32 = mybir.dt.float32
BF16 = mybir.dt.bfloat16


def build_nc():
    """Build the per-core Bass graph (SPMD: same graph on all 8 cores)."""
    nc = bacc.Bacc(None, target_bir_lowering=False)

    x = nc.declare_dram_parameter("x", [B_SHARD, C, HW], BF16, isOutput=False)
    consts = nc.declare_dram_parameter("consts", [P, CONST_COLS], F32, isOutput=False)
    out = nc.declare_dram_parameter("out", [B_SHARD, C, HW], BF16, isOutput=True)

    with tile.TileContext(nc) as tc:
        with (
            tc.tile_pool(name="const", bufs=1) as constp,
            tc.tile_pool(name="boxp", bufs=1) as boxp,
            tc.tile_pool(name="maskp", bufs=1) as maskp,
            tc.tile_pool(name="xp", bufs=4) as xp,
            tc.tile_pool(name="outp", bufs=2) as outp,
            tc.tile_pool(name="pbsp", bufs=2) as pbsp,
            tc.tile_pool(name="adp", bufs=2) as adp,
            tc.tile_pool(name="smallp", bufs=2) as smallp,
            tc.tile_pool(name="ps_s", bufs=2, space=bass.MemorySpace.PSUM) as ps_s,
            tc.tile_pool(name="ps_b", bufs=3, space=bass.MemorySpace.PSUM) as ps_b,
            tc.tile_pool(name="ps_m", bufs=1, space=bass.MemorySpace.PSUM) as ps_m,
        ):
            # --- constants: one small DMA on the scalar ring ---
            cst = constp.tile([P, CONST_COLS], F32)
            nc.scalar.dma_start(cst[:], consts[:])
            iota_f = cst[:, 0:P]
            wsum = constp.tile([P, 1], BF16)  # 1/C column -> channel mean
            nc.vector.memset(wsum[:], ALPHA / C)
            ones1 = constp.tile([1, P], BF16)  # K=1 broadcast row
            nc.vector.memset(ones1[:], 1.0)

            # --- box rasterization setup (tiny) ---
            # Per box n (on partitions): vx1m1 = (xc-bw/2)*W - 1, vx2 = (xc+bw/2)*W
            # cols[n,w] = (w > vx1m1) & (w <= vx2)   (== ref's clamped-int test)
            # valid = (#cols>=2) & (#rows>=2)        (== ref's x2>x1 & y2>y1)
            rows_sel = [[None] * N_BOX_TILES for _ in range(B_SHARD)]
            cols_val = [None] * N_BOX_TILES
            rows_raw, sel_tiles = [], []
            for t in range(N_BOX_TILES):
                bx = cst[:, P + 4 * t : P + 4 * (t + 1)]
                st = cst[:, P + 4 * N_BOX_TILES + 2 * t : P + 4 * N_BOX_TILES + 2 * (t + 1)]

                xc, yc, bw, bh = (bx[:, i : i + 1] for i in range(4))

                def lohi(center, ext, scale, tag):
                    """[P,1] box edges: lo-1 = c*s - ext*s/2 - 1, hi = c*s + ext*s/2"""
                    cs1 = smallp.tile([P, 1], F32, tag=tag + "c1")
                    nc.vector.tensor_scalar(
                        cs1[:], center, float(scale), -1.0,
                        op0=mybir.AluOpType.mult, op1=mybir.AluOpType.add,
                    )
                    he = smallp.tile([P, 1], F32, tag=tag + "he")
                    nc.vector.tensor_scalar_mul(he[:], ext, scale / 2.0)
                    lo = smallp.tile([P, 1], F32, tag=tag + "lo")
                    nc.vector.tensor_sub(lo[:], cs1[:], he[:])
                    hi = smallp.tile([P, 1], F32, tag=tag + "hi")
                    nc.vector.scalar_tensor_tensor(
                        hi[:], center, float(scale), he[:],
                        op0=mybir.AluOpType.mult, op1=mybir.AluOpType.add,
                    )
                    return lo, hi

                vx1m1, vx2 = lohi(xc, bw, W, f"x{t}")
                vy1m1, vy2 = lohi(yc, bh, H, f"y{t}")

                def member(lo_m1, hi, tag):
                    """m[n,w] = (w > lo-1) & (w <= hi); cnt[n] = row count"""
                    g2 = smallp.tile([P, P], F32, tag=tag + "g2")
                    nc.vector.tensor_scalar(
                        g2[:], iota_f, hi[:], None, op0=mybir.AluOpType.is_le
                    )
                    m = boxp.tile([P, P], F32, tag=tag + "m")
                    cnt = smallp.tile([P, 1], F32, tag=tag + "cnt")
                    nc.vector.scalar_tensor_tensor(
                        m[:], iota_f, lo_m1[:], g2[:],
                        op0=mybir.AluOpType.is_gt, op1=mybir.AluOpType.mult,
                        accum_out=cnt[:],
                    )
                    return m, cnt

                cols, ccnt = member(vx1m1, vx2, f"c{t}")
                rows, rcnt = member(vy1m1, vy2, f"r{t}")

                # valid = (#cols>=2)&(#rows>=2); fold into cols
                cok = smallp.tile([P, 1], F32, tag=f"cok{t}")
                nc.vector.tensor_scalar(
                    cok[:], ccnt[:], 1.5, None, op0=mybir.AluOpType.is_ge
                )
                vfac = smallp.tile([P, 1], F32, tag=f"vfac{t}")
                nc.vector.scalar_tensor_tensor(
                    vfac[:], rcnt[:], 1.5, cok[:],
                    op0=mybir.AluOpType.is_ge, op1=mybir.AluOpType.mult,
                )
                cv = boxp.tile([P, P], F32, tag=f"cv{t}")
                nc.vector.tensor_scalar(
                    cv[:], cols[:], vfac[:], None, op0=mybir.AluOpType.mult
                )
                cols_val[t] = cv
                rows_raw.append(rows)
                sel_tiles.append(st)

            # --- per-image mask -> flat [1, HW] bf16 on partition 0 via a
            # small HWDGE flatten on the scalar ring. Image 0 first so the
            # main stream unblocks early.
            mflat = []
            for j in range(B_SHARD):
                for t in range(N_BOX_TILES):
                    rs = boxp.tile([P, P], F32, tag=f"rs{t}_{j}")
                    nc.vector.tensor_scalar(
                        rs[:], rows_raw[t][:], sel_tiles[t][:, j : j + 1], None,
                        op0=mybir.AluOpType.mult,
                    )
                    rows_sel[j][t] = rs
                pm = ps_m.tile([P, W], F32)
                for t in range(N_BOX_TILES):
                    nc.tensor.matmul(
                        pm[:], rows_sel[j][t][:], cols_val[t][:],
                        start=(t == 0), stop=(t == N_BOX_TILES - 1),
                    )
                msb = maskp.tile([P, W], BF16, tag=f"msb{j}")
                nc.vector.tensor_scalar_min(msb[:], pm[:], 1.0)
                mf = maskp.tile([1, HW], BF16, tag=f"mf{j}")
                nc.scalar.dma_start(mf[:], msb[:])
                mflat.append(mf)

            # --- main stream: 8 chunks of [256, 4096], software-pipelined.
            # One 2 MiB load per chunk on the sync HWDGE ring (loads only:
            # no store waits can head-of-line-block the prefetch); one 2 MiB
            # store per chunk via SWDGE on the Pool queue (store waits only
            # block other stores). X/O tiles are [128, 2*CHUNK]: columns
            # 0:CHUNK = channels 0-127, CHUNK:2*CHUNK = channels 128-255.
            def emit_front(b, ci):
                csl = slice(ci * CHUNK, (ci + 1) * CHUNK)
                X = xp.tile([P, 2 * CHUNK], BF16, tag="X")
                nc.sync.dma_start(
                    X[:], x[b, :, csl].rearrange("(h p) w -> p h w", h=2)
                )
                # channel sums: one long same-stationary PE streak
                pss, ads = [], []
                for sp in range(N_PAIR):
                    ps = ps_s.tile([1, PAIR], F32)
                    for h in range(2):
                        ssl = slice((sp * 2 + h) * SUB, (sp * 2 + h + 1) * SUB)
                        bsl = slice(
                            CHUNK + (sp * 2 + h) * SUB, CHUNK + (sp * 2 + h + 1) * SUB
                        )
                        hsl = slice(h * SUB, (h + 1) * SUB)
                        nc.tensor.matmul(
                            ps[:, hsl], wsum[:], X[:, ssl], start=True, stop=False
                        )
                        nc.tensor.matmul(
                            ps[:, hsl], wsum[:], X[:, bsl], start=False, stop=True
                        )
                    pss.append(ps)
                    # masked means (frees the ps slot for pair sp+2)
                    ad = adp.tile([1, PAIR], BF16, tag="ad")
                    moff = ci * CHUNK + sp * PAIR
                    nc.vector.tensor_mul(
                        ad[:], ps[:], mflat[b][0:1, moff : moff + PAIR]
                    )
                    ads.append(ad)
                # broadcasts: second same-stationary PE streak; ACT converts
                pbs = pbsp.tile([P, CHUNK], BF16, tag="pbs")
                for sp in range(N_PAIR):
                    for h in range(2):
                        pb = ps_b.tile([P, SUB], F32)
                        nc.tensor.matmul(
                            pb[:], ones1[:], ads[sp][:, h * SUB : (h + 1) * SUB],
                            start=True, stop=True,
                        )
                        psl = slice((sp * 2 + h) * SUB, (sp * 2 + h + 1) * SUB)
                        nc.scalar.copy(pbs[:, psl], pb[:])
                return (b, ci, X, pbs)

            def emit_back(st):
                b, ci, X, pbs = st
                csl = slice(ci * CHUNK, (ci + 1) * CHUNK)
                O = outp.tile([P, 2 * CHUNK], BF16, tag="O")
                nc.vector.tensor_add(O[:, 0:CHUNK], X[:, 0:CHUNK], pbs[:])
                nc.vector.tensor_add(O[:, CHUNK : 2 * CHUNK], X[:, CHUNK : 2 * CHUNK], pbs[:])
                nc.gpsimd.dma_start(
                    out[b, :, csl].rearrange("(h p) w -> p h w", h=2), O[:]
                )

            prev = None
            for b in range(B_SHARD):
                for ci in range(N_CHUNK):
                    cur = emit_front(b, ci)
                    if prev is not None:
                        emit_back(prev)
                    prev = cur
            emit_back(prev)

    return nc


def _host_prep(x, bboxes, batch_idx):
    """Shard inputs; cast x to bf16; build the packed const array."""
    x = (
        np.ascontiguousarray(np.asarray(x, dtype=np.float32))
        .reshape(B, C, HW)
        .astype(ml_dtypes.bfloat16)
    )
    bboxes = np.asarray(bboxes, dtype=np.float32)
    batch_idx = np.asarray(batch_idx).astype(np.int64)

    boxes_pad = np.zeros((NBOX_PAD, 4), dtype=np.float32)
    boxes_pad[:N_BOX] = bboxes
    # [128, 12]: partition p, tile t -> box t*128+p
    boxes_cols = boxes_pad.reshape(N_BOX_TILES, P, 4).transpose(1, 0, 2).reshape(P, -1)
    iota = np.broadcast_to(np.arange(P, dtype=np.float32), (P, P))

    in_maps = []
    for i in range(N_CORES):
        sel_i = np.zeros((NBOX_PAD, 2), dtype=np.float32)
        for j in range(B_SHARD):
            sel_i[:N_BOX, j] = (batch_idx == (i * B_SHARD + j)).astype(np.float32)
        sel_cols = sel_i.reshape(N_BOX_TILES, P, 2).transpose(1, 0, 2).reshape(P, -1)
        consts = np.concatenate([iota, boxes_cols, sel_cols], axis=1).astype(np.float32)
        in_maps.append(
            {
                "x": np.ascontiguousarray(x[i * B_SHARD : (i + 1) * B_SHARD]),
                "consts": np.ascontiguousarray(consts),
            }
        )
    return in_maps


def kernel(x, bboxes, batch_idx):
    in_maps = _host_prep(x, bboxes, batch_idx)
    nc = build_nc()
    nc.finalize()
    res = bass_utils.run_bass_kernel_spmd(nc, in_maps, core_ids=list(range(N_CORES)))
    shards = [
        np.asarray(res.results[i]["out"]).astype(np.float32) for i in range(N_CORES)
    ]
    return np.concatenate(shards, axis=0).reshape(B, C, H, W)


if __name__ == "__main__":
    nc = build_nc()
    nc.finalize()
    print("built ok:", len(nc.inst_map), "instructions")
